# revision 75
# baseline (speedup 1.0000x reference)
"""Trainium2 Bass kernel for nn_Clip_OCR_Block (OCR attention block).

Sharding: 8 cores; core j handles image n=j//2, spatial half h=j%2
(8192 of 16384 pixels). The SpatialTemporalGather proxy needs a
full-image spatial reduction -> each core computes partial proxy
numerator/denominator over its half and pair-AllReduces with its
sibling core. Everything else is pixel-local.

Mixed precision: bf16 activations/weights with f32 PSUM accumulation;
the q1/q2 chain runs fp8e4m3 DoubleRow (0.5 cyc/row, contraction 256
per matmul; weights host-scaled x16, undone by ACT scale=1/16) — the
attention softmax downstream tolerates the ~5% fp8 noise since logits
span only ~0.45. rel-err ~3.5e-3 vs the 2e-2 gate. BN scales fold into
weights/biases on the host. Softmaxes skip max-subtraction: |probs| <=
~5.5 and attention logits are tiny, so exp never overflows.

Key structural choices (host prep is layout/dtype/folding only):
 - feats arrive once in [c,s] bf16 (resident in SBUF all kernel) plus
   an fp8 copy for the DoubleRow q1; the [s,c] layout for the proxy
   contraction is produced on-chip (PE transposes into the B2-idle
   psY banks, DVE copies), keeping B1's DMA at ~12.7 MiB.
 - q2 stays resident in SBUF: no DRAM spill/reload.
 - f_up is folded through the attention: ctx2 = ReLU(V2 @ sim + bu)
   with V2 = Wu_folded @ val^T (ctx = sim @ val is rank-K), removing
   the f_up conv (1.07 GMAC/core) and ctx matmul entirely.
 - attention logits are computed transposed ([px,k], ap=20 matmuls
   with q2 chunks stationary), so the K-softmax runs on ACT/DVE along
   the free dim (no PE denominator/broadcast matmuls); the normalized
   sim is PE-transposed back (ident stationary) for the V2 matmul.

Phases (one Tile graph; engines overlap via dependencies):
  A0: probsT -> exp -> eT [s,k] resident; den = ones^T @ eT chunks.
  B1 (16 tiles of 512 px): DMA F + F8; transpose F -> fT on-chip;
      proxy num accumulation (stationary fT chunks, ap=20); fp8
      DoubleRow q1(t-1)/q2(t-2) staggered. Last DEFER_T chains
      deferred past the AllReduce issue.
  Mid: pair-AllReduce proxy num+den (50 KB); normalize; kk/val tiny
      convs; V2T = val^T-stationary x Wu-moving.
  B2 (16 tiles): logitsT -> softmax (ACT/DVE) -> simT -> ctx2 via V2
      -> final conv on [ctx2 | F] -> out store (bf16, host casts).
"""
import numpy as np
import ml_dtypes

import concourse.bacc as bacc
import concourse.mybir as mybir
import concourse.tile as tile
from concourse.bass_utils import run_bass_kernel_spmd

f32 = mybir.dt.float32
bf16 = mybir.dt.bfloat16
f8 = mybir.dt.float8e4
AF = mybir.ActivationFunctionType

N, C, H, W = 4, 512, 128, 128
K, KC, OUT = 19, 256, 512
HW = H * W
HALF = HW // 2            # 8192 pixels per core
NCH = HALF // 128         # 64 chunks of 128 px
NT = HALF // 512          # 16 s-tiles of 512 px
SCALE = KC ** -0.5
KP = 20                   # K padded for alignment
DEFER_T = 4               # q-chains deferred past the AllReduce issue

# bias column layout in the packed [128, 18] f32 bias tensor
BP1, BP2, BO1, BO2, BD, BU, BF, B16 = 0, 2, 4, 6, 8, 10, 14, 18

_CACHED = {}

# When True, _build_nc replaces the pair-AllReduce with a local DRAM copy so
# the module has no collectives and can run under TimelineSim (timing proxy).
_SIM_NO_COLLECTIVE = False


def _build_nc():
    nc = bacc.Bacc("TRN2", target_bir_lowering=False, debug=False, num_devices=8)

    fC_d = nc.dram_tensor("fC", [128, NT, 4, 512], bf16, kind="ExternalInput")
    f8_d = nc.dram_tensor("f8s", [128, NT, 4, 512], f8, kind="ExternalInput")
    # probs stay f32 into the exp: bf16-quantizing pre-exp values is a
    # systematic (pixel-correlated) perturbation of the softmax weights.
    pT_d = nc.dram_tensor("pT", [128, NCH, KP], f32, kind="ExternalInput")
    # q-chain weights (wp1|wp2) and the rest (wo1|wo2|wd|wu|wf), each packed
    # into one tensor so descriptor generation isn't serialized per weight
    # fp8e4m3, host-scaled x16 (undone by ACT scale=1/16), packed in
    # DoubleRow pair order: q1 pair j,o at [j*4+o*2 : +2], q2 at [8+o*2 : +2]
    wpq_d = nc.dram_tensor("wpq", [128, 12, 128], f8, kind="ExternalInput")
    wrest_d = nc.dram_tensor("wrest", [128, 60, 128], bf16,
                             kind="ExternalInput")
    bias_d = nc.dram_tensor("biases", [128, 20], f32, kind="ExternalInput")
    consts_d = nc.dram_tensor("consts", [128, 256], bf16, kind="ExternalInput")
    out_d = nc.dram_tensor("out_half", [128, NT, 4, 512], bf16,
                           kind="ExternalOutput")

    # proxy num [c-part, 4 c-chunks, KP] + den on [0:1, 4, :]
    prox_in = nc.dram_tensor("prox_in", [128, 5, KP], f32)
    prox_out = nc.dram_tensor("prox_out", [128, 5, KP], f32)

    with tile.TileContext(nc) as tc:
        with nc.allow_low_precision(reason="bf16 compute, 2e-2 gate"), \
             tc.tile_pool(name="w", bufs=1) as wp, \
             tc.tile_pool(name="b", bufs=2) as bp, \
             tc.tile_pool(name="psA", bufs=1, space="PSUM") as ppA, \
             tc.tile_pool(name="psQ", bufs=2, space="PSUM") as ppQ, \
             tc.tile_pool(name="psT", bufs=1, space="PSUM") as ppT, \
             tc.tile_pool(name="psY", bufs=2, space="PSUM") as ppY, \
             tc.tile_pool(name="psF", bufs=2, space="PSUM") as ppF:

            # ---- A0 input first: exp gates the proxy chain ----
            pt = wp.tile([128, NCH, KP], f32, tag="pt")
            nc.sync.dma_start(pt[:], pT_d.ap())

            # ---- PE warmup: dummy matmuls while the first DMAs land ----
            scratch = wp.tile([128, 512], bf16, tag="scratch")
            nc.vector.memset(scratch[:], 0.0)
            for i in range(6):
                ps_w = ppQ.tile([128, 512], f32, tag="q", name="ps_warm")
                nc.tensor.matmul(ps_w[:], scratch[:, :128], scratch[:],
                                 start=True, stop=True)

            # ---- persistent consts / weights / biases ----
            consts = wp.tile([128, 256], bf16, tag="consts")
            nc.sync.dma_start(consts[:], consts_d.ap())
            ident = consts[:, 0:128]
            bias = wp.tile([128, 20], f32, tag="bias")
            nc.sync.dma_start(bias[:], bias_d.ap())

            wpq = wp.tile([128, 12, 128], f8, tag="wpq")
            nc.sync.dma_start(wpq[:], wpq_d.ap())

            def wp1dr(j, o):
                return wpq[:, j * 4 + o * 2:j * 4 + o * 2 + 2, :]

            def wp2dr(o):
                return wpq[:, 8 + o * 2:8 + o * 2 + 2, :]

            # ---- A0: probsT -> exp -> eT [s,k]; den via ones-stationary ----
            eT = wp.tile([128, NCH, KP], bf16, tag="eT")
            ps_a = ppA.tile([128, 5, KP], f32, tag="a", name="ps_prox")
            # den group-partials live in the psT slot (idle until mid-phase)
            ps_dg = ppT.tile([1, 16, KP], f32, tag="tr", name="ps_den")
            for g in range(4):
                c0 = g * 16
                nc.scalar.activation(eT[:, c0:c0 + 16, :], pt[:, c0:c0 + 16, :],
                                     AF.Exp)
                nc.tensor.matmul(ps_dg[:], consts[:, 128:129],
                                 eT[:, c0:c0 + 16, :],
                                 start=(g == 0), stop=(g == 3))

            # ---- resident activations ----
            F = wp.tile([128, NT, 4, 512], bf16, tag="F")
            q2r = wp.tile([128, 2, HALF], bf16, tag="q2r")
            # final-conv feat-half partials for the first SPILL_T tiles,
            # computed in B1 (where PE idles on DMA) and added back in B2
            SPILL_T = 0
            spill = None
            if SPILL_T:
                spill = wp.tile([128, SPILL_T, 4, 512], bf16, tag="spill",
                                name="spill")

            # q-chain in fp8e4m3 DoubleRow (0.5 cyc/row, contraction 256
            # per matmul). Weights are x16 on the host; ACT scale=1/16
            # undoes it. The attention softmax downstream tolerates the
            # ~5% fp8 quantization noise (logits span only ~0.45).
            f8s = {}

            def q1part(t, on_dve=False):
                q1 = bp.tile([128, 2, 512], f8, tag="q1", bufs=4, name="q1")
                f8t = f8s[t]
                for o in range(2):
                    ps = ppQ.tile([128, 512], f32, tag="q", name="ps_q1")
                    for j in range(2):
                        nc.tensor.matmul(
                            ps[:], wp1dr(j, o), f8t[:, 2 * j:2 * j + 2, :],
                            start=(j == 0), stop=(j == 1),
                            perf_mode=mybir.MatmulPerfMode.DoubleRow)
                    if on_dve:
                        # deferred chains: ReLU+bias on DVE so the ACT queue
                        # doesn't serialize the mid-phase entry. Stores 16*q1
                        # (Relu(16x+16b) = 16 Relu(x+b), bias16 cols hold
                        # 16*b1); q2part undoes it via scale=1/256.
                        nc.vector.scalar_tensor_tensor(
                            q1[:, o, :], ps[:],
                            bias[:, B16 + o:B16 + o + 1], scratch[:],
                            op0=mybir.AluOpType.add,
                            op1=mybir.AluOpType.max)
                    else:
                        nc.scalar.activation(q1[:, o, :], ps[:], AF.Relu,
                                             bias=bias[:, BP1 + o:BP1 + o + 1],
                                             scale=1.0 / 16)
                return q1

            def q2part(t, q1, qsc=1.0 / 16):
                for o in range(2):
                    ps = ppQ.tile([128, 512], f32, tag="q", name="ps_q2")
                    nc.tensor.matmul(
                        ps[:], wp2dr(o), q1[:],
                        start=True, stop=True,
                        perf_mode=mybir.MatmulPerfMode.DoubleRow)
                    nc.scalar.activation(q2r[:, o, t * 512:(t + 1) * 512],
                                         ps[:], AF.Relu,
                                         bias=bias[:, BP2 + o:BP2 + o + 1],
                                         scale=qsc)

            def wo1v(k, o):
                return wrest[:, k * 2 + o, :]

            def wo2v(k, o):
                return wrest[:, 8 + k * 2 + o, :]

            def wdv(k, o):
                return wrest[:, 12 + k * 2 + o, :]

            def wuv(k, o):
                return wrest[:, 20 + k * 4 + o, :]

            def wfv(k, o):
                return wrest[:, 28 + k * 4 + o, :]

            def spill_group(g):
                s, o = g // 4, g % 4
                ps = ppQ.tile([128, 512], f32, tag="q", name="ps_sp")
                for i, k in enumerate((4, 5, 6, 7)):
                    nc.tensor.matmul(ps[:], wfv(k, o), F[:, s, k - 4, :],
                                     start=(i == 0), stop=(i == 3))
                nc.vector.tensor_copy(spill[:, s, o, :], ps[:])

            # Staggered pipeline: q1(t-1) and q2(t-2) per iteration — q2
            # reads a q1 whose activation finished an iteration ago, so the
            # PE stream never stalls on the ACT ping-pong.
            wrest = None
            q1s = {}
            NCHAIN = NT - DEFER_T  # chains 0..NCHAIN-1 inline
            for t in range(NT):
                nc.sync.dma_start(F[:, t], fC_d[:, t])
                # fp8 feats for the DoubleRow q1, streamed from the host
                # (gpsimd copies proved 2.7 us each in the cost model).
                # bufs=8 keeps the deferred chains' tiles alive.
                f8t = bp.tile([128, 4, 512], f8, tag="F8", bufs=8, name="f8t")
                nc.sync.dma_start(f8t[:], f8_d[:, t])
                f8s[t] = f8t
                if t == 2:
                    wrest = wp.tile([128, 60, 128], bf16, tag="wrest")
                    nc.sync.dma_start(wrest[:], wrest_d.ap())
                # fT [s,c] produced on-chip: PE transposes (ident moving)
                # into the B2-idle psY banks, DVE copies to SBUF. Saves the
                # 8 MiB fS stream that made B1 DMA-bound.
                ft = bp.tile([128, 4, 512], bf16, tag="fT", bufs=3, name="ft")
                for half in range(2):
                    ps_tr = ppY.tile([128, 2, 4, 128], bf16, tag="y",
                                     name="ps_ftr")
                    for ha in range(2):
                        a = half * 2 + ha
                        for c in range(4):
                            nc.tensor.transpose(
                                ps_tr[:, ha, c, :],
                                F[:, t, c, a * 128:(a + 1) * 128], ident)
                    nc.vector.tensor_copy(
                        ft[:, half * 2:half * 2 + 2, :],
                        ps_tr[:].rearrange("p h c m -> p h (c m)"))
                if 1 <= t and t - 1 < NCHAIN:
                    q1s[t - 1] = q1part(t - 1)
                if 2 <= t and t - 2 < NCHAIN:
                    q2part(t - 2, q1s.pop(t - 2))
                for a in range(4):
                    tt = t * 4 + a
                    for c in range(4):
                        nc.tensor.matmul(
                            ps_a[:, c, :],
                            ft[:, a, c * 128:(c + 1) * 128], eT[:, tt, :],
                            start=(tt == 0), stop=(tt == NCH - 1))
                # one final-conv feat-half o-group per iteration, spread so
                # the in-order PE stream never bulges past the DMA cadence
                if 3 <= t and t - 3 < 4 * SPILL_T:
                    spill_group(t - 3)
            if NCHAIN - 1 in q1s:
                q2part(NCHAIN - 1, q1s.pop(NCHAIN - 1))

            for g in range(NT - 3, 4 * SPILL_T):
                spill_group(g)

            # ---- Mid: AllReduce proxy, normalize, kk/val/V2T ----
            prox_sb = wp.tile([128, 5, KP], f32, tag="proxsb")
            nc.vector.memset(prox_sb[:], 0.0)
            nc.vector.tensor_copy(prox_sb[:, 0:4, :], ps_a[:, 0:4, :])
            nc.vector.tensor_reduce(prox_sb[0:1, 4, :],
                                    ps_dg[:].rearrange("p g k -> p k g"),
                                    axis=mybir.AxisListType.X,
                                    op=mybir.AluOpType.add)
            nc.sync.dma_start(prox_in[:], prox_sb[:])
            if _SIM_NO_COLLECTIVE:
                nc.sync.dma_start(prox_out[:], prox_in[:])
            else:
                nc.gpsimd.collective_compute(
                    "AllReduce", mybir.AluOpType.add,
                    replica_groups=[[0, 1], [2, 3], [4, 5], [6, 7]],
                    ins=[prox_in[:]], outs=[prox_out[:]])

            # load B2 weights + run deferred q-chains while AllReduce flies
            red = wp.tile([128, 5, KP], f32, tag="red")
            nc.sync.dma_start(red[:], prox_out[:])
            # deferred chains interleaved across tiles: all q1 stages first,
            # then all q2 stages, so ACT latencies hide behind other tiles' mms
            dq1s = [q1part(t) for t in range(NT - DEFER_T, NT)]
            for i, t in enumerate(range(NT - DEFER_T, NT)):
                q2part(t, dq1s[i])

            rden_bf = wp.tile([1, KP], bf16, tag="rdenbf")
            nc.vector.reciprocal(rden_bf[:], red[0:1, 4, :])
            ps_bc = ppY.tile([128, 512], f32, tag="y", name="ps_bc")
            nc.tensor.matmul(ps_bc[:, :KP], consts[0:1, 128:256], rden_bf[:],
                             start=True, stop=True)
            prox_n = wp.tile([128, 4, KP], bf16, tag="proxn")
            for c in range(4):
                nc.vector.tensor_mul(prox_n[:, c, :], red[:, c, :],
                                     ps_bc[:, :KP])

            def sconv(wt, bcol, rhs, kin, kout, tag):
                res = wp.tile([128, kout, KP], bf16, tag=tag)
                for o in range(kout):
                    ps = ppQ.tile([128, 512], f32, tag="q", name="ps_sc")
                    for k in range(kin):
                        nc.tensor.matmul(ps[:, :KP], wt(k, o),
                                         rhs[:, k, :],
                                         start=(k == 0), stop=(k == kin - 1))
                    nc.scalar.activation(res[:, o, :], ps[:, :KP], AF.Relu,
                                         bias=bias[:, bcol + o:bcol + o + 1],
                                         scale=1.0)
                return res

            kk1 = sconv(wo1v, BO1, prox_n, 4, 2, "kk1")
            kk = sconv(wo2v, BO2, kk1, 2, 2, "kk")
            val = sconv(wdv, BD, prox_n, 4, 2, "val")

            ps_v = ppT.tile([KP, 4, 128], f32, tag="tr", name="ps_v")
            for oc in range(4):
                for j in range(2):
                    nc.tensor.matmul(ps_v[:, oc, :], val[:, j, :],
                                     wuv(j, oc),
                                     start=(j == 0), stop=(j == 1))
            V2T = wp.tile([KP, 4, 128], bf16, tag="V2T")
            nc.vector.tensor_copy(V2T[:], ps_v[:])

            # ---- B2: attention + final conv, software-pipelined ----
            st = [dict() for _ in range(NT)]

            def att_logits(t):
                d = st[t]
                ps_lg = ppA.tile([128, 4, KP], f32, tag="a", name="ps_lg")
                for pc in range(4):
                    sl = slice(t * 512 + pc * 128, t * 512 + (pc + 1) * 128)
                    for j in range(2):
                        nc.tensor.matmul(ps_lg[:, pc, :], q2r[:, j, sl],
                                         kk[:, j, :],
                                         start=(j == 0), stop=(j == 1))
                eatt = bp.tile([128, 4, KP], bf16, tag="eatt", name="eatt")
                nc.scalar.activation(eatt[:], ps_lg[:], AF.Exp, scale=SCALE)
                d["eatt"] = eatt

            def att_soft(t):
                d = st[t]
                den = bp.tile([128, 4], f32, tag="den", name="den")
                nc.vector.tensor_reduce(den[:], d["eatt"][:, :, 0:K],
                                        axis=mybir.AxisListType.X,
                                        op=mybir.AluOpType.add)
                rc = bp.tile([128, 4], f32, tag="rc", name="rc")
                nc.vector.reciprocal(rc[:], den[:])
                sim = bp.tile([128, 4, KP], bf16, tag="sim", name="sim")
                for c in range(4):
                    nc.vector.tensor_scalar_mul(sim[:, c, :],
                                                in0=d["eatt"][:, c, :],
                                                scalar1=rc[:, c:c + 1])
                d["sim"] = sim

            def att_tr(t):
                d = st[t]
                ps_tr = ppT.tile([KP, 4, 128], bf16, tag="tr", name="ps_tr")
                for c in range(4):
                    nc.tensor.transpose(ps_tr[:, c, :], d["sim"][:, c, :],
                                        ident)
                simT = bp.tile([KP, 4, 128], bf16, tag="simT", name="simT")
                nc.vector.tensor_copy(simT[:], ps_tr[:])
                d["simT"] = simT

            def att_y(t):
                d = st[t]
                ctx2 = bp.tile([128, 4, 512], bf16, tag="ctx2", bufs=3,
                               name="ctx2")
                for oc in range(4):
                    ps = ppY.tile([128, 512], f32, tag="y", name="ps_y")
                    nc.tensor.matmul(ps[:], V2T[:K, oc, :], d["simT"][:K],
                                     start=True, stop=True)
                    nc.scalar.activation(ctx2[:, oc, :], ps[:], AF.Relu,
                                         bias=bias[:, BU + oc:BU + oc + 1],
                                         scale=1.0)
                d["ctx2"] = ctx2

            def fin_feat(t, os_):
                if t < SPILL_T:
                    return  # feat half precomputed in B1 (spill)
                d = st[t]
                ps_map = d.setdefault("ps", {})
                for o in os_:
                    ps = ppF.tile([128, 512], f32, tag="f", name="ps_f")
                    ps_map[o] = ps
                    for i, k in enumerate((4, 5, 6, 7)):
                        nc.tensor.matmul(ps[:], wfv(k, o),
                                         F[:, t, k - 4, :],
                                         start=(i == 0), stop=False)

            def fin_ctx(t, os_):
                d = st[t]
                if "ot" not in d:
                    d["ot"] = bp.tile([128, 4, 512], bf16, tag="ot", bufs=3,
                                      name="ot")
                ot = d["ot"]
                spilled = t < SPILL_T
                for o in os_:
                    if spilled:
                        ps = ppF.tile([128, 512], f32, tag="f", name="ps_f")
                    else:
                        ps = d["ps"].pop(o)
                    for i, k in enumerate((0, 1, 2, 3)):
                        nc.tensor.matmul(ps[:], wfv(k, o),
                                         d["ctx2"][:, k, :],
                                         start=spilled and (i == 0),
                                         stop=(i == 3))
                    if spilled:
                        nc.vector.tensor_add(ps[:], ps[:], spill[:, t, o, :])
                    nc.scalar.activation(ot[:, o, :], ps[:], AF.Relu,
                                         bias=bias[:, BF + o:BF + o + 1],
                                         scale=1.0)
                if t == NT - 1:
                    # tail: ship each o-pair as soon as its ACTs land
                    nc.sync.dma_start(out_d[:, t, os_[0]:os_[-1] + 1],
                                      ot[:, os_[0]:os_[-1] + 1, :])
                    if os_[-1] == 3:
                        st[t] = None
                elif os_[-1] == 3:
                    nc.sync.dma_start(out_d[:, t], ot[:])
                    st[t] = None

            # depth-3 pipeline: fin works on tile t-2 while tile t runs its
            # attention chain — latency of the att chain (ACT/DVE hops) is
            # covered even on spilled tiles whose PE volume is small
            for t in range(NT + 2):
                if t < NT:
                    att_logits(t)
                if t >= 2:
                    fin_feat(t - 2, (0, 1))
                if t < NT:
                    att_soft(t)
                    att_tr(t)
                if t >= 2:
                    fin_ctx(t - 2, (0, 1))
                if t < NT:
                    att_y(t)
                if t >= 2:
                    fin_feat(t - 2, (2, 3))
                    fin_ctx(t - 2, (2, 3))

    nc.compile()
    return nc


def _fold(w, b, s, t):
    """conv+BN fold: y = s*(Wx+b)+t = (s.W)x + (s*b+t)."""
    w = np.asarray(w, np.float32)
    b = np.asarray(b, np.float32)
    s = np.asarray(s, np.float32)
    t = np.asarray(t, np.float32)
    return (s[:, None] * w), (s * b + t)


def _wprep(wT, kin, kout):
    """[in, out] f32 -> [128, kin, kout, 128] bf16 (DMA-ready)."""
    a = wT.reshape(kin, 128, kout, 128).transpose(1, 0, 2, 3)
    return np.ascontiguousarray(a.astype(ml_dtypes.bfloat16))


def kernel(feats, probs,
           wp1, bp1, sp1, tp1, wp2, bp2, sp2, tp2,
           wo1, bo1, so1, to1, wo2, bo2, so2, to2,
           wd, bd, sd, td, wu, bu, su, tu,
           wf, bf, sf, tf, clip_num, _trace=False):
    feats = np.ascontiguousarray(np.asarray(feats, np.float32))
    probs = np.ascontiguousarray(np.asarray(probs, np.float32))

    W1, B1 = _fold(wp1, bp1, sp1, tp1)
    W2, B2 = _fold(wp2, bp2, sp2, tp2)
    WO1, BO1v = _fold(wo1, bo1, so1, to1)
    WO2, BO2v = _fold(wo2, bo2, so2, to2)
    WD, BDv = _fold(wd, bd, sd, td)
    WU, BUv = _fold(wu, bu, su, tu)
    WF, BFv = _fold(wf, bf, sf, tf)

    biases = np.zeros((128, 20), np.float32)
    for col, vec in ((BP1, B1), (BP2, B2), (BO1, BO1v), (BO2, BO2v),
                     (BD, BDv), (BU, BUv), (BF, BFv), (B16, 16.0 * B1)):
        n = vec.size // 128
        biases[:, col:col + n] = vec.reshape(n, 128).T

    consts = np.zeros((128, 256), np.float32)
    consts[:, 0:128] = np.eye(128)
    consts[:, 128:256] = 1.0

    def _flat(wT, kin, kout):
        return _wprep(wT, kin, kout).reshape(128, kin * kout, 128)

    def _drpack(wT, kin, kout):
        """[in,out] f32 -> [128, kin*kout, 128] fp8e4m3 in DoubleRow pair
        order: flat index (j, o, i) with k = 2j+i the contraction chunk."""
        a = (16.0 * wT).reshape(kin // 2, 2, 128, kout, 128)  # j i p o m
        a = a.transpose(2, 0, 3, 1, 4).reshape(128, kin * kout, 128)
        return np.ascontiguousarray(a.astype(ml_dtypes.float8_e4m3))

    wpq = np.concatenate([_drpack(W1.T, 4, 2), _drpack(W2.T, 2, 2)], axis=1)
    wrest = np.concatenate([
        _flat(WO1.T, 4, 2), _flat(WO2.T, 2, 2), _flat(WD.T, 4, 2),
        _flat(WU.T, 2, 4), _flat(WF.T, 8, 4)], axis=1)
    shared = {
        "wpq": np.ascontiguousarray(wpq),
        "wrest": np.ascontiguousarray(wrest),
        "biases": biases,
        "consts": np.ascontiguousarray(consts.astype(ml_dtypes.bfloat16)),
    }

    fr = feats.reshape(N, C, HW)
    pr = probs.reshape(N, K, HW)
    in_maps = []
    for j in range(8):
        n, h = j // 2, j % 2
        sl = slice(h * HALF, (h + 1) * HALF)
        fc = fr[n, :, sl]                              # [512, 8192] f32
        fC = fc.reshape(4, 128, NT, 512).transpose(1, 2, 0, 3)
        ptp = np.zeros((HALF, KP), np.float32)
        ptp[:, :K] = pr[n, :, sl].T
        pT = ptp.reshape(NCH, 128, KP).transpose(1, 0, 2)
        in_maps.append({
            "fC": np.ascontiguousarray(fC.astype(ml_dtypes.bfloat16)),
            "f8s": np.ascontiguousarray(fC.astype(ml_dtypes.float8_e4m3)),
            "pT": np.ascontiguousarray(pT, dtype=np.float32),
            **shared,
        })

    if "nc" not in _CACHED:
        _CACHED["nc"] = _build_nc()
    nc = _CACHED["nc"]

    res = run_bass_kernel_spmd(nc, in_maps, list(range(8)), trace=_trace)
    out = np.empty((N, OUT, HW), np.float32)
    for j in range(8):
        n, h = j // 2, j % 2
        buf = np.asarray(res.results[j]["out_half"], dtype=np.float32)
        # [128, NT, 4, 512] -> [o, p, t, s] -> [512, 8192]
        out[n, :, h * HALF:(h + 1) * HALF] = (
            buf.transpose(2, 0, 1, 3).reshape(OUT, HALF))
    if _trace:
        kernel.last_exec_time_ns = res.exec_time_ns
        kernel.last_results = res
    return out.reshape(N, OUT, H, W)
